# revision 4
# baseline (speedup 1.0000x reference)
"""Trainium2 Bass kernel for causal self-attention with GQA + RoPE.

Problem: B=2, T=2048, D=2048, H=16 q-heads, HKV=4 kv-heads, dk=128, causal,
RoPE (interleaved-pair), out = softmax(QK^T/sqrt(dk) + mask) V @ c_w.T.

Sharding (8 NeuronCores): 4 head-groups x 2 batches. Each core handles one
batch and 4 q-heads / 1 kv-head, computes a partial c_proj output (its head
group's contribution, transposed), and the host reduces over head groups.

Device-side dataflow per core (all matmuls fp32r, 1 cyc/row at N>=512):
  phase 1: Q^T/K^T/V^T projections with weight chunks stationary and x
    moving (x read once). RoPE applied in the transposed [feat, t] layout
    using an even/odd weight-row permutation: features are pre-permuted to
    [64 even | 64 odd] blocks, duplicated-partition copies E2=[E;E], O2=[O;O]
    are built with SB<-PSUM DMAs (the only partition remapper), and a single
    mul/mul/add against stacked cos/sin tables produces the rotated rows at
    full 128-lane DVE width. V^T is PE-transposed back to natural [t, dk]
    for use as the PV stationary operand.
  phase 2: per 512-wide q block: S^T tile = (K^T chunk).T @ Q^T (logits
    transposed, kpos on partitions), exp on ACT with the 1/sqrt(dk) scale
    folded in (no max subtraction: logits are ~N(0,1) so exp cannot
    overflow), PV and a ones-row matmul accumulate O^T and the softmax
    denominator in PSUM, normalize, then c_proj^T and DMA out. Causality is
    exploited by slicing the valid column range per diagonal tile; only the
    triangular 128x128 block needs an additive mask.
"""
import math
import sys

sys.path.insert(0, "/opt/trn_rl_repo")

import numpy as np

D_MODEL = 2048
N_HEADS = 16
N_KV_HEADS = 4
ROPE_THETA = 10000.0
DK = 128
B, T = 2, 2048
G = 4           # head groups (= kv heads); one q-head group = 4 heads = 512 feats
HG = N_HEADS // G
P = 128
KC = D_MODEL // P          # 16 contraction chunks
TQ = 4                     # 512-wide q/t blocks
NT = T // P                # 16 t tiles
SCALE = 1.0 / math.sqrt(DK)

_CACHE = {}


def _build_bass():
    import concourse.mybir as mybir
    import concourse.tile as tile
    from concourse import bacc
    from concourse.bass import ts

    dtf = mybir.dt.float32
    dtr = mybir.dt.float32r
    FT = mybir.ActivationFunctionType

    nc = bacc.Bacc("TRN2", target_bir_lowering=False, debug=False, num_devices=8)

    x3 = nc.declare_dram_parameter("x3", [P, KC, T], dtr, isOutput=False)
    qw3 = nc.declare_dram_parameter("qw3", [P, KC, HG * DK], dtr, isOutput=False)
    kw3 = nc.declare_dram_parameter("kw3", [P, KC, DK], dtr, isOutput=False)
    vw3 = nc.declare_dram_parameter("vw3", [P, KC, DK], dtr, isOutput=False)
    cw3 = nc.declare_dram_parameter("cw3", [P, HG, D_MODEL], dtr, isOutput=False)
    c2 = nc.declare_dram_parameter("c2", [P, T], dtf, isOutput=False)
    s2 = nc.declare_dram_parameter("s2", [P, T], dtf, isOutput=False)
    trimask = nc.declare_dram_parameter("trimask", [P, P], dtf, isOutput=False)
    ident = nc.declare_dram_parameter("ident", [P, P], dtf, isOutput=False)
    onesw = nc.declare_dram_parameter("onesw", [P, P], dtr, isOutput=False)
    outT = nc.declare_dram_parameter("outT", [D_MODEL, T], dtf, isOutput=True)

    with tile.TileContext(nc) as tc:
        with (
            tc.tile_pool(name="wres", bufs=1) as wres,
            tc.tile_pool(name="acts", bufs=1) as acts,
        ):
            qw_sb = wres.tile([P, KC, HG * DK], dtr, tag="qw")
            kw_sb = wres.tile([P, KC, DK], dtr, tag="kw")
            vw_sb = wres.tile([P, KC, DK], dtr, tag="vw")
            nc.sync.dma_start(qw_sb[:], qw3[:])
            nc.sync.dma_start(kw_sb[:], kw3[:])
            nc.sync.dma_start(vw_sb[:], vw3[:])

            QT = acts.tile([P, HG, T], dtr, tag="QT")   # rotated Q^T per head
            KT = acts.tile([P, T], dtr, tag="KT")       # rotated K^T
            V = acts.tile([P, NT, DK], dtr, tag="V")    # V natural [t, dk]

            # ---------------- phase 1: projections + RoPE + V transpose ----
            with (
                tc.tile_pool(name="p1c", bufs=1) as p1c,
                tc.tile_pool(name="xs", bufs=4) as xs,
                tc.tile_pool(name="ps1", bufs=1, space="PSUM") as ps1,
                tc.tile_pool(name="vps", bufs=1, space="PSUM") as vps,
                tc.tile_pool(name="qn", bufs=2) as qnp,
                tc.tile_pool(name="eo", bufs=2) as eo,
                tc.tile_pool(name="rt", bufs=2) as rt,
                tc.tile_pool(name="vst", bufs=2) as vst,
            ):
                c2_sb = p1c.tile([P, T], dtf, tag="c2")
                s2_sb = p1c.tile([P, T], dtf, tag="s2")
                id_sb = p1c.tile([P, P], dtf, tag="id")
                nc.sync.dma_start(c2_sb[:], c2[:])
                nc.sync.dma_start(s2_sb[:], s2[:])
                nc.sync.dma_start(id_sb[:], ident[:])

                for tq in range(TQ):
                    pss = [ps1.tile([P, 512], dtf, tag=f"ps{m}", name=f"ps{m}") for m in range(6)]
                    for kc in range(KC):
                        xt = xs.tile([P, 512], dtr, tag="x")
                        nc.sync.dma_start(xt[:], x3[:, kc, ts(tq, 512)])
                        for m in range(6):  # 0-3 q heads, 4 k, 5 v
                            if m < HG:
                                w = qw_sb[:, kc, ts(m, DK)]
                            elif m == HG:
                                w = kw_sb[:, kc, :]
                            else:
                                w = vw_sb[:, kc, :]
                            nc.tensor.matmul(
                                pss[m][:], w, xt[:],
                                start=(kc == 0), stop=(kc == KC - 1),
                            )
                    for m in range(6):
                        if m <= HG:
                            qn = qnp.tile([P, 512], dtf, tag="qn")
                            nc.scalar.copy(qn[:], pss[m][:])
                            e2 = eo.tile([P, 512], dtf, tag="e2")
                            o2 = eo.tile([P, 512], dtf, tag="o2")
                            nc.sync.dma_start(e2[0:64, :], qn[0:64, :])
                            nc.sync.dma_start(e2[64:128, :], qn[0:64, :])
                            nc.sync.dma_start(o2[0:64, :], qn[64:128, :])
                            nc.sync.dma_start(o2[64:128, :], qn[64:128, :])
                            t1 = rt.tile([P, 512], dtf, tag="t1")
                            t2 = rt.tile([P, 512], dtf, tag="t2")
                            nc.vector.tensor_mul(t1[:], e2[:], c2_sb[:, ts(tq, 512)])
                            nc.vector.tensor_mul(t2[:], o2[:], s2_sb[:, ts(tq, 512)])
                            dest = QT[:, m, ts(tq, 512)] if m < HG else KT[:, ts(tq, 512)]
                            nc.vector.tensor_add(dest, t1[:], t2[:])
                        else:
                            vt = vst.tile([P, 512], dtf, tag="vt")
                            nc.scalar.copy(vt[:], pss[m][:])
                            for i in range(4):
                                tp = vps.tile([P, P], dtf, tag="vtp")
                                nc.tensor.transpose(tp[:], vt[:, ts(i, P)], id_sb[:])
                                nc.scalar.copy(V[:, tq * 4 + i, :], tp[:])

            # ---------------- phase 2: attention + c_proj ------------------
            with (
                tc.tile_pool(name="p2c", bufs=1) as p2c,
                tc.tile_pool(name="stp", bufs=2, space="PSUM") as stp,
                tc.tile_pool(name="otp", bufs=2, space="PSUM") as otp,
                tc.tile_pool(name="rsp", bufs=2, space="PSUM") as rsp,
                tc.tile_pool(name="cpp", bufs=2, space="PSUM") as cpp,
                tc.tile_pool(name="ptp", bufs=6) as ptp,
                tc.tile_pool(name="smp", bufs=3) as smp,
                tc.tile_pool(name="ytp", bufs=8) as ytp,
                tc.tile_pool(name="rcpp", bufs=3) as rcpp,
                tc.tile_pool(name="obp", bufs=3) as obp,
            ):
                cw_sb = p2c.tile([P, HG, D_MODEL], dtr, tag="cw")
                tm_sb = p2c.tile([P, P], dtf, tag="tm")
                on_sb = p2c.tile([P, P], dtr, tag="on")
                nc.sync.dma_start(cw_sb[:], cw3[:])
                nc.sync.dma_start(tm_sb[:], trimask[:])
                nc.sync.dma_start(on_sb[:], onesw[:])
                for qb in range(TQ):
                    yts = []
                    for h in range(HG):
                        nj = 4 * (qb + 1)
                        ot = otp.tile([P, 512], dtf, tag="ot")
                        rs = rsp.tile([P, 512], dtf, tag="rs")
                        for j in range(nj):
                            d = j - qb * 4
                            lo = d * P if d >= 0 else 0
                            st = stp.tile([P, 512], dtf, tag="st")
                            nc.tensor.matmul(
                                st[:, lo:512],
                                KT[:, ts(j, P)],
                                QT[:, h, qb * 512 + lo:(qb + 1) * 512],
                                start=True, stop=True,
                            )
                            pt = ptp.tile([P, 512], dtr, tag="pt")
                            if d >= 0:
                                sm = smp.tile([P, P], dtf, tag="sm")
                                nc.vector.tensor_add(sm[:], st[:, lo:lo + P], tm_sb[:])
                                nc.scalar.activation(
                                    pt[:, lo:lo + P], sm[:], FT.Exp, scale=SCALE
                                )
                                if lo + P < 512:
                                    nc.scalar.activation(
                                        pt[:, lo + P:512], st[:, lo + P:512],
                                        FT.Exp, scale=SCALE,
                                    )
                            else:
                                nc.scalar.activation(pt[:], st[:], FT.Exp, scale=SCALE)
                            nc.tensor.matmul(
                                ot[:, lo:512], V[:, j, :], pt[:, lo:512],
                                start=(j == 0), stop=(j == nj - 1),
                            )
                            nc.tensor.matmul(
                                rs[:, lo:512], on_sb[:], pt[:, lo:512],
                                start=(j == 0), stop=(j == nj - 1),
                            )
                        rcp = rcpp.tile([P, 512], dtf, tag="rcp")
                        nc.vector.reciprocal(rcp[:], rs[:])
                        yt = ytp.tile([P, 512], dtr, tag="yt")
                        nc.vector.tensor_mul(yt[:], ot[:], rcp[:])
                        yts.append(yt)
                    for otile in range(16):
                        cp = cpp.tile([P, 512], dtf, tag="cp")
                        for h in range(HG):
                            nc.tensor.matmul(
                                cp[:], cw_sb[:, h, ts(otile, P)], yts[h][:],
                                start=(h == 0), stop=(h == HG - 1),
                            )
                        ob = obp.tile([P, 512], dtf, tag="ob")
                        nc.any.tensor_copy(ob[:], cp[:])
                        nc.sync.dma_start(outT[ts(otile, P), ts(qb, 512)], ob[:])

    nc.compile()
    return nc


def _get_nc():
    if "nc" not in _CACHE:
        _CACHE["nc"] = _build_bass()
    return _CACHE["nc"]


def _prep_core_inputs(x, q_w, kv_w, c_w):
    """Build the 8 per-core input dicts (core = 2*g + b)."""
    perm = np.concatenate([np.arange(0, P, 2), np.arange(1, P, 2)])  # even|odd

    # RoPE tables, stacked for the [64 even | 64 odd] block layout.
    pairs = np.arange(DK // 2, dtype=np.float64)
    freqs = 1.0 / ROPE_THETA ** (2.0 * pairs / DK)
    pos = np.arange(T, dtype=np.float64)
    ang = pos[None, :] * freqs[:, None]            # [64, T]
    cos = np.cos(ang)
    sin = np.sin(ang)
    c2 = np.concatenate([cos, sin], axis=0).astype(np.float32)      # [128, T]
    s2 = np.concatenate([-sin, cos], axis=0).astype(np.float32)     # [128, T]

    cc = np.arange(P)[:, None]
    qq = np.arange(P)[None, :]
    trimask = np.where(cc <= qq, 0.0, -1e30).astype(np.float32)
    ident = np.eye(P, dtype=np.float32)
    onesw = np.ones((P, P), dtype=np.float32)

    def chunk3(a2d):
        # [Dfull, F] -> [128, Dfull//128, F]
        d_full, f = a2d.shape
        return np.ascontiguousarray(
            a2d.reshape(d_full // P, P, f).transpose(1, 0, 2)
        )

    in_maps = []
    for core in range(8):
        g, b = core // 2, core % 2
        q_w_g = q_w[g * HG * DK:(g + 1) * HG * DK, :]          # [512, D]
        q_w_gp = q_w_g.reshape(HG, DK, D_MODEL)[:, perm, :].reshape(HG * DK, D_MODEL)
        k_w_g = kv_w[g * DK:(g + 1) * DK, :][perm, :]          # [128, D]
        v_w_g = kv_w[N_KV_HEADS * DK + g * DK: N_KV_HEADS * DK + (g + 1) * DK, :]
        c_w_g = c_w[:, g * HG * DK:(g + 1) * HG * DK]          # [D, 512]

        in_maps.append({
            "x3": chunk3(np.ascontiguousarray(x[b].T)),        # [128,16,T]
            "qw3": chunk3(np.ascontiguousarray(q_w_gp.T)),     # [128,16,512]
            "kw3": chunk3(np.ascontiguousarray(k_w_g.T)),      # [128,16,128]
            "vw3": chunk3(np.ascontiguousarray(v_w_g.T)),      # [128,16,128]
            "cw3": chunk3(np.ascontiguousarray(c_w_g.T)),      # [128,4,2048]
            "c2": c2, "s2": s2, "trimask": trimask,
            "ident": ident, "onesw": onesw,
        })
    return in_maps


def kernel(x, q_w, kv_w, c_w, _trace=False, _trace_cores=None):
    from concourse.bass_utils import run_bass_kernel_spmd

    nc = _get_nc()
    in_maps = _prep_core_inputs(
        np.asarray(x, dtype=np.float32), np.asarray(q_w, dtype=np.float32),
        np.asarray(kv_w, dtype=np.float32), np.asarray(c_w, dtype=np.float32),
    )
    res = run_bass_kernel_spmd(
        nc, in_maps, core_ids=list(range(8)),
        trace=_trace, trace_cores=_trace_cores,
    )
    _CACHE["last_results"] = res

    out = np.zeros((B, T, D_MODEL), dtype=np.float32)
    for core in range(8):
        g, b = core // 2, core % 2
        out[b] += res.results[core]["outT"].T
    return out


# revision 8
# speedup vs baseline: 1.0258x; 1.0258x over previous
"""Trainium2 Bass kernel for causal self-attention with GQA + RoPE.

Problem: B=2, T=2048, D=2048, H=16 q-heads, HKV=4 kv-heads, dk=128, causal,
RoPE (interleaved-pair), out = softmax(QK^T/sqrt(dk) + mask) V @ c_w.T.

Sharding (8 NeuronCores): 4 head-groups x 2 batches. Each core handles one
batch and 4 q-heads / 1 kv-head, computes a partial c_proj output (its head
group's contribution, transposed), and the host reduces over head groups.

Device-side dataflow per core (all matmuls fp32r, 1 cyc/row at N>=512):
  phase 1: Q^T/K^T/V^T projections with weight chunks stationary and x
    moving (x read once). RoPE applied in the transposed [feat, t] layout
    using an even/odd weight-row permutation: features are pre-permuted to
    [64 even | 64 odd] blocks, duplicated-partition copies E2=[E;E], O2=[O;O]
    are built with SB<-PSUM DMAs (the only partition remapper), and a single
    mul/mul/add against stacked cos/sin tables produces the rotated rows at
    full 128-lane DVE width. V^T is PE-transposed back to natural [t, dk]
    for use as the PV stationary operand.
  phase 2: per 512-wide q block: S^T tile = (K^T chunk).T @ Q^T (logits
    transposed, kpos on partitions), exp on ACT with the 1/sqrt(dk) scale
    folded in (no max subtraction: logits are ~N(0,1) so exp cannot
    overflow), PV and a ones-row matmul accumulate O^T and the softmax
    denominator in PSUM, normalize, then c_proj^T and DMA out. Causality is
    exploited by slicing the valid column range per diagonal tile; only the
    triangular 128x128 block needs an additive mask.
"""
import math
import sys

sys.path.insert(0, "/opt/trn_rl_repo")

import numpy as np

D_MODEL = 2048
N_HEADS = 16
N_KV_HEADS = 4
ROPE_THETA = 10000.0
DK = 128
B, T = 2, 2048
G = 4           # head groups (= kv heads); one q-head group = 4 heads = 512 feats
HG = N_HEADS // G
P = 128
KC = D_MODEL // P          # 16 contraction chunks
TQ = 4                     # 512-wide q/t blocks
NT = T // P                # 16 t tiles
SCALE = 1.0 / math.sqrt(DK)

_CACHE = {}


def _build_bass():
    import concourse.mybir as mybir
    import concourse.tile as tile
    from concourse import bacc
    from concourse.bass import ts

    dtf = mybir.dt.float32
    dtr = mybir.dt.float32r
    FT = mybir.ActivationFunctionType

    nc = bacc.Bacc("TRN2", target_bir_lowering=False, debug=False, num_devices=8)

    x3 = nc.declare_dram_parameter("x3", [P, KC, T], dtr, isOutput=False)
    qw3 = nc.declare_dram_parameter("qw3", [P, KC, HG * DK], dtr, isOutput=False)
    kw3 = nc.declare_dram_parameter("kw3", [P, KC, DK], dtr, isOutput=False)
    vw3 = nc.declare_dram_parameter("vw3", [P, KC, DK], dtr, isOutput=False)
    cw3 = nc.declare_dram_parameter("cw3", [P, HG, D_MODEL], dtr, isOutput=False)
    c2 = nc.declare_dram_parameter("c2", [P, T], dtf, isOutput=False)
    s2 = nc.declare_dram_parameter("s2", [P, T], dtf, isOutput=False)
    trimask = nc.declare_dram_parameter("trimask", [P, P], dtf, isOutput=False)
    ident = nc.declare_dram_parameter("ident", [P, P], dtf, isOutput=False)
    onesw = nc.declare_dram_parameter("onesw", [P, P], dtr, isOutput=False)
    outT = nc.declare_dram_parameter("outT", [D_MODEL, T], dtf, isOutput=True)

    with tile.TileContext(nc) as tc:
        with (
            tc.tile_pool(name="wres", bufs=1) as wres,
            tc.tile_pool(name="acts", bufs=1) as acts,
        ):
            qw_sb = wres.tile([P, KC, HG * DK], dtr, tag="qw")
            kw_sb = wres.tile([P, KC, DK], dtr, tag="kw")
            vw_sb = wres.tile([P, KC, DK], dtr, tag="vw")
            cw_sb = wres.tile([P, HG, D_MODEL], dtr, tag="cw")
            tm_sb = wres.tile([P, P], dtf, tag="tm")
            on_sb = wres.tile([P, P], dtr, tag="on")
            # chunked so the first matmuls only wait on their own chunk
            for kc in range(KC):
                nc.sync.dma_start(qw_sb[:, kc, :], qw3[:, kc, :])
                nc.sync.dma_start(kw_sb[:, kc, :], kw3[:, kc, :])
                nc.sync.dma_start(vw_sb[:, kc, :], vw3[:, kc, :])
            for fc in range(HG):
                nc.sync.dma_start(cw_sb[:, fc, :], cw3[:, fc, :])
            nc.sync.dma_start(tm_sb[:], trimask[:])
            nc.sync.dma_start(on_sb[:], onesw[:])

            QT = acts.tile([P, HG, T], dtr, tag="QT")   # rotated Q^T per head
            KT = acts.tile([P, T], dtr, tag="KT")       # rotated K^T
            V = acts.tile([P, NT, DK], dtr, tag="V")    # V natural [t, dk]

            # ---------------- phase 1: projections + RoPE + V transpose ----
            with (
                tc.tile_pool(name="p1c", bufs=1) as p1c,
                tc.tile_pool(name="xs", bufs=4) as xs,
                tc.tile_pool(name="ps1", bufs=1, space="PSUM") as ps1,
                tc.tile_pool(name="vps", bufs=1, space="PSUM") as vps,
                tc.tile_pool(name="qn", bufs=2) as qnp,
                tc.tile_pool(name="eo", bufs=2) as eo,
                tc.tile_pool(name="rt", bufs=2) as rt,
                tc.tile_pool(name="vst", bufs=2) as vst,
            ):
                c2_sb = p1c.tile([P, T], dtf, tag="c2")
                s2_sb = p1c.tile([P, T], dtf, tag="s2")
                id_sb = p1c.tile([P, P], dtf, tag="id")
                nc.sync.dma_start(c2_sb[:], c2[:])
                nc.sync.dma_start(s2_sb[:], s2[:])
                nc.sync.dma_start(id_sb[:], ident[:])

                for tq in range(TQ):
                    pss = [ps1.tile([P, 512], dtf, tag=f"ps{m}", name=f"ps{m}") for m in range(6)]
                    for kc in range(KC):
                        xt = xs.tile([P, 512], dtr, tag="x")
                        nc.sync.dma_start(xt[:], x3[:, kc, ts(tq, 512)])
                        for m in range(6):  # 0-3 q heads, 4 k, 5 v
                            if m < HG:
                                w = qw_sb[:, kc, ts(m, DK)]
                            elif m == HG:
                                w = kw_sb[:, kc, :]
                            else:
                                w = vw_sb[:, kc, :]
                            nc.tensor.matmul(
                                pss[m][:], w, xt[:],
                                start=(kc == 0), stop=(kc == KC - 1),
                            )
                    for m in range(6):
                        if m <= HG:
                            qn = qnp.tile([P, 512], dtf, tag="qn")
                            if m % 2 == 0:
                                nc.scalar.copy(qn[:], pss[m][:])
                            else:
                                nc.vector.tensor_copy(qn[:], pss[m][:])
                            e2 = eo.tile([P, 512], dtf, tag="e2")
                            o2 = eo.tile([P, 512], dtf, tag="o2")
                            nc.sync.dma_start(e2[0:64, :], qn[0:64, :])
                            nc.sync.dma_start(e2[64:128, :], qn[0:64, :])
                            nc.sync.dma_start(o2[0:64, :], qn[64:128, :])
                            nc.sync.dma_start(o2[64:128, :], qn[64:128, :])
                            t1 = rt.tile([P, 512], dtf, tag="t1")
                            t2 = rt.tile([P, 512], dtf, tag="t2")
                            nc.vector.tensor_mul(t1[:], e2[:], c2_sb[:, ts(tq, 512)])
                            nc.vector.tensor_mul(t2[:], o2[:], s2_sb[:, ts(tq, 512)])
                            dest = QT[:, m, ts(tq, 512)] if m < HG else KT[:, ts(tq, 512)]
                            nc.vector.tensor_add(dest, t1[:], t2[:])
                        else:
                            vt = vst.tile([P, 512], dtf, tag="vt")
                            nc.scalar.copy(vt[:], pss[m][:])
                            for i in range(4):
                                tp = vps.tile([P, P], dtf, tag="vtp")
                                nc.tensor.transpose(tp[:], vt[:, ts(i, P)], id_sb[:])
                                nc.scalar.copy(V[:, tq * 4 + i, :], tp[:])

            # ---------------- phase 2: attention + c_proj ------------------
            with (
                tc.tile_pool(name="stp", bufs=2, space="PSUM") as stp,
                tc.tile_pool(name="otp", bufs=2, space="PSUM") as otp,
                tc.tile_pool(name="rsp", bufs=2, space="PSUM") as rsp,
                tc.tile_pool(name="cpp", bufs=2, space="PSUM") as cpp,
                tc.tile_pool(name="ptp", bufs=6) as ptp,
                tc.tile_pool(name="smp", bufs=3) as smp,
                tc.tile_pool(name="ytp", bufs=8) as ytp,
                tc.tile_pool(name="rcpp", bufs=3) as rcpp,
                tc.tile_pool(name="obp", bufs=3) as obp,
            ):
                for qb in range(TQ):
                    yts = []
                    for h in range(HG):
                        nj = 4 * (qb + 1)
                        ot = otp.tile([P, 512], dtf, tag="ot")
                        rs = rsp.tile([P, 512], dtf, tag="rs")
                        for j in range(nj):
                            d = j - qb * 4
                            lo = d * P if d >= 0 else 0
                            st = stp.tile([P, 512], dtf, tag="st")
                            nc.tensor.matmul(
                                st[:, lo:512],
                                KT[:, ts(j, P)],
                                QT[:, h, qb * 512 + lo:(qb + 1) * 512],
                                start=True, stop=True,
                            )
                            pt = ptp.tile([P, 512], dtr, tag="pt")
                            if d >= 0:
                                sm = smp.tile([P, P], dtf, tag="sm")
                                nc.vector.tensor_add(sm[:], st[:, lo:lo + P], tm_sb[:])
                                nc.scalar.activation(
                                    pt[:, lo:lo + P], sm[:], FT.Exp, scale=SCALE
                                )
                                if lo + P < 512:
                                    nc.scalar.activation(
                                        pt[:, lo + P:512], st[:, lo + P:512],
                                        FT.Exp, scale=SCALE,
                                    )
                            else:
                                nc.scalar.activation(pt[:], st[:], FT.Exp, scale=SCALE)
                            nc.tensor.matmul(
                                ot[:, lo:512], V[:, j, :], pt[:, lo:512],
                                start=(j == 0), stop=(j == nj - 1),
                            )
                            nc.tensor.matmul(
                                rs[:, lo:512], on_sb[:], pt[:, lo:512],
                                start=(j == 0), stop=(j == nj - 1),
                            )
                        rcp = rcpp.tile([P, 512], dtf, tag="rcp")
                        nc.vector.reciprocal_approx_fast(rcp[:], rs[:])
                        yt = ytp.tile([P, 512], dtr, tag="yt")
                        nc.vector.tensor_mul(yt[:], ot[:], rcp[:])
                        yts.append(yt)
                    for otile in range(16):
                        cp = cpp.tile([P, 512], dtf, tag="cp")
                        for h in range(HG):
                            nc.tensor.matmul(
                                cp[:], cw_sb[:, h, ts(otile, P)], yts[h][:],
                                start=(h == 0), stop=(h == HG - 1),
                            )
                        ob = obp.tile([P, 512], dtf, tag="ob")
                        nc.any.tensor_copy(ob[:], cp[:])
                        nc.sync.dma_start(outT[ts(otile, P), ts(qb, 512)], ob[:])

    nc.compile()
    return nc


def _get_nc():
    if "nc" not in _CACHE:
        _CACHE["nc"] = _build_bass()
    return _CACHE["nc"]


def _prep_core_inputs(x, q_w, kv_w, c_w):
    """Build the 8 per-core input dicts (core = 2*g + b)."""
    perm = np.concatenate([np.arange(0, P, 2), np.arange(1, P, 2)])  # even|odd

    # RoPE tables, stacked for the [64 even | 64 odd] block layout.
    pairs = np.arange(DK // 2, dtype=np.float64)
    freqs = 1.0 / ROPE_THETA ** (2.0 * pairs / DK)
    pos = np.arange(T, dtype=np.float64)
    ang = pos[None, :] * freqs[:, None]            # [64, T]
    cos = np.cos(ang)
    sin = np.sin(ang)
    c2 = np.concatenate([cos, sin], axis=0).astype(np.float32)      # [128, T]
    s2 = np.concatenate([-sin, cos], axis=0).astype(np.float32)     # [128, T]

    cc = np.arange(P)[:, None]
    qq = np.arange(P)[None, :]
    trimask = np.where(cc <= qq, 0.0, -1e30).astype(np.float32)
    ident = np.eye(P, dtype=np.float32)
    onesw = np.ones((P, P), dtype=np.float32)

    def chunk3(a2d):
        # [Dfull, F] -> [128, Dfull//128, F]
        d_full, f = a2d.shape
        return np.ascontiguousarray(
            a2d.reshape(d_full // P, P, f).transpose(1, 0, 2)
        )

    in_maps = []
    for core in range(8):
        g, b = core // 2, core % 2
        q_w_g = q_w[g * HG * DK:(g + 1) * HG * DK, :]          # [512, D]
        q_w_gp = q_w_g.reshape(HG, DK, D_MODEL)[:, perm, :].reshape(HG * DK, D_MODEL)
        k_w_g = kv_w[g * DK:(g + 1) * DK, :][perm, :]          # [128, D]
        v_w_g = kv_w[N_KV_HEADS * DK + g * DK: N_KV_HEADS * DK + (g + 1) * DK, :]
        c_w_g = c_w[:, g * HG * DK:(g + 1) * HG * DK]          # [D, 512]

        in_maps.append({
            "x3": chunk3(np.ascontiguousarray(x[b].T)),        # [128,16,T]
            "qw3": chunk3(np.ascontiguousarray(q_w_gp.T)),     # [128,16,512]
            "kw3": chunk3(np.ascontiguousarray(k_w_g.T)),      # [128,16,128]
            "vw3": chunk3(np.ascontiguousarray(v_w_g.T)),      # [128,16,128]
            "cw3": chunk3(np.ascontiguousarray(c_w_g.T)),      # [128,4,2048]
            "c2": c2, "s2": s2, "trimask": trimask,
            "ident": ident, "onesw": onesw,
        })
    return in_maps


def kernel(x, q_w, kv_w, c_w, _trace=False, _trace_cores=None):
    from concourse.bass_utils import run_bass_kernel_spmd

    nc = _get_nc()
    in_maps = _prep_core_inputs(
        np.asarray(x, dtype=np.float32), np.asarray(q_w, dtype=np.float32),
        np.asarray(kv_w, dtype=np.float32), np.asarray(c_w, dtype=np.float32),
    )
    res = run_bass_kernel_spmd(
        nc, in_maps, core_ids=list(range(8)),
        trace=_trace, trace_cores=_trace_cores,
    )
    _CACHE["last_results"] = res

    out = np.zeros((B, T, D_MODEL), dtype=np.float32)
    for core in range(8):
        g, b = core // 2, core % 2
        out[b] += res.results[core]["outT"].T
    return out


# revision 12
# speedup vs baseline: 1.0918x; 1.0644x over previous
"""Trainium2 Bass kernel for causal self-attention with GQA + RoPE.

Problem: B=2, T=2048, D=2048, H=16 q-heads, HKV=4 kv-heads, dk=128, causal,
RoPE (interleaved-pair), out = softmax(QK^T/sqrt(dk) + mask) V @ c_w.T.

Sharding (8 NeuronCores): 4 head-groups x 2 batches. Each core handles one
batch and 4 q-heads / 1 kv-head, computes a partial c_proj output (its head
group's contribution, transposed), and the host reduces over head groups.

Device-side dataflow per core (all matmuls fp32r, 1 cyc/row at N>=512):
  phase 1: Q^T/K^T/V^T projections with weight chunks stationary and x
    moving (x read once). RoPE applied in the transposed [feat, t] layout
    using an even/odd weight-row permutation: features are pre-permuted to
    [64 even | 64 odd] blocks, duplicated-partition copies E2=[E;E], O2=[O;O]
    are built with SB<-PSUM DMAs (the only partition remapper), and a single
    mul/mul/add against stacked cos/sin tables produces the rotated rows at
    full 128-lane DVE width. V^T is PE-transposed back to natural [t, dk]
    for use as the PV stationary operand.
  phase 2: per 512-wide q block: S^T tile = (K^T chunk).T @ Q^T (logits
    transposed, kpos on partitions), exp on ACT with the 1/sqrt(dk) scale
    folded in (no max subtraction: logits are ~N(0,1) so exp cannot
    overflow), PV and a ones-row matmul accumulate O^T and the softmax
    denominator in PSUM, normalize, then c_proj^T and DMA out. Causality is
    exploited by slicing the valid column range per diagonal tile; only the
    triangular 128x128 block needs an additive mask.
"""
import math
import sys

sys.path.insert(0, "/opt/trn_rl_repo")

import numpy as np

D_MODEL = 2048
N_HEADS = 16
N_KV_HEADS = 4
ROPE_THETA = 10000.0
DK = 128
B, T = 2, 2048
G = 4           # head groups (= kv heads); one q-head group = 4 heads = 512 feats
HG = N_HEADS // G
P = 128
KC = D_MODEL // P          # 16 contraction chunks
TQ = 4                     # 512-wide q/t blocks
NT = T // P                # 16 t tiles
SCALE = 1.0 / math.sqrt(DK)

_CACHE = {}


def _build_bass():
    import concourse.mybir as mybir
    import concourse.tile as tile
    from concourse import bacc
    from concourse.bass import ts

    dtf = mybir.dt.float32
    dtr = mybir.dt.float32r
    FT = mybir.ActivationFunctionType

    nc = bacc.Bacc("TRN2", target_bir_lowering=False, debug=False, num_devices=8)

    x3 = nc.declare_dram_parameter("x3", [P, KC, T], dtr, isOutput=False)
    qw3 = nc.declare_dram_parameter("qw3", [P, KC, HG * DK], dtr, isOutput=False)
    kw3 = nc.declare_dram_parameter("kw3", [P, KC, DK], dtr, isOutput=False)
    vw3 = nc.declare_dram_parameter("vw3", [P, KC, DK], dtr, isOutput=False)
    cw3 = nc.declare_dram_parameter("cw3", [P, HG, D_MODEL], dtr, isOutput=False)
    c2 = nc.declare_dram_parameter("c2", [P, T], dtf, isOutput=False)
    s2 = nc.declare_dram_parameter("s2", [P, T], dtf, isOutput=False)
    trimask = nc.declare_dram_parameter("trimask", [P, P], dtf, isOutput=False)
    ident = nc.declare_dram_parameter("ident", [P, P], dtf, isOutput=False)
    onesw = nc.declare_dram_parameter("onesw", [P, P], dtr, isOutput=False)
    outT = nc.declare_dram_parameter("outT", [D_MODEL, T], dtf, isOutput=True)

    with tile.TileContext(nc) as tc:
        with (
            tc.tile_pool(name="wres", bufs=1) as wres,
            tc.tile_pool(name="acts", bufs=1) as acts,
        ):
            qw_sb = wres.tile([P, KC, HG * DK], dtr, tag="qw")
            kw_sb = wres.tile([P, KC, DK], dtr, tag="kw")
            vw_sb = wres.tile([P, KC, DK], dtr, tag="vw")
            cw_sb = wres.tile([P, HG, D_MODEL], dtr, tag="cw")
            tm_sb = wres.tile([P, P], dtf, tag="tm")
            on_sb = wres.tile([P, P], dtr, tag="on")
            # chunked so the first matmuls only wait on their own chunk;
            # issued from the gpsimd queue to keep the sync sequencer free
            # for the x-stream (each dma_start costs ~0.6us of sequencer
            # time, so issue order and engine split matter).
            for kc in range(KC):
                nc.gpsimd.dma_start(qw_sb[:, kc, :], qw3[:, kc, :])
                nc.gpsimd.dma_start(kw_sb[:, kc, :], kw3[:, kc, :])
                nc.gpsimd.dma_start(vw_sb[:, kc, :], vw3[:, kc, :])
            for fc in range(HG):
                for oc in range(4):
                    nc.gpsimd.dma_start(
                        cw_sb[:, fc, ts(oc, 512)], cw3[:, fc, ts(oc, 512)]
                    )
            nc.gpsimd.dma_start(tm_sb[:], trimask[:])
            nc.gpsimd.dma_start(on_sb[:], onesw[:])

            QT = acts.tile([P, HG, T], dtr, tag="QT")   # rotated Q^T per head
            KT = acts.tile([P, T], dtr, tag="KT")       # rotated K^T
            V = acts.tile([P, NT, DK], dtr, tag="V")    # V natural [t, dk]

            # ---------------- phase 1: projections + RoPE + V transpose ----
            with (
                tc.tile_pool(name="p1c", bufs=1) as p1c,
                tc.tile_pool(name="xs", bufs=4) as xs,
                tc.tile_pool(name="ps1", bufs=1, space="PSUM") as ps1,
                tc.tile_pool(name="vps", bufs=1, space="PSUM") as vps,
                tc.tile_pool(name="qn", bufs=2) as qnp,
                tc.tile_pool(name="eo", bufs=2) as eo,
                tc.tile_pool(name="rt", bufs=2) as rt,
                tc.tile_pool(name="vst", bufs=2) as vst,
            ):
                c2_sb = p1c.tile([P, T], dtf, tag="c2")
                s2_sb = p1c.tile([P, T], dtf, tag="s2")
                id_sb = p1c.tile([P, P], dtf, tag="id")
                nc.sync.dma_start(c2_sb[:], c2[:])
                nc.sync.dma_start(s2_sb[:], s2[:])
                nc.sync.dma_start(id_sb[:], ident[:])

                for tq in range(TQ):
                    pss = [ps1.tile([P, 512], dtf, tag=f"ps{m}", name=f"ps{m}") for m in range(6)]
                    for kc in range(KC):
                        xt = xs.tile([P, 512], dtr, tag="x")
                        nc.sync.dma_start(xt[:], x3[:, kc, ts(tq, 512)])
                        for m in range(6):  # 0-3 q heads, 4 k, 5 v
                            if m < HG:
                                w = qw_sb[:, kc, ts(m, DK)]
                            elif m == HG:
                                w = kw_sb[:, kc, :]
                            else:
                                w = vw_sb[:, kc, :]
                            nc.tensor.matmul(
                                pss[m][:], w, xt[:],
                                start=(kc == 0), stop=(kc == KC - 1),
                            )
                    for m in range(6):
                        if m <= HG:
                            qn = qnp.tile([P, 512], dtf, tag="qn")
                            # psum drain is the release path for the next tq's
                            # matmuls -> keep it early in the engine streams
                            with tc.high_priority(offset=200):
                                if m % 2 == 0:
                                    nc.scalar.copy(qn[:], pss[m][:])
                                else:
                                    nc.vector.tensor_copy(qn[:], pss[m][:])
                            e2 = eo.tile([P, 512], dtf, tag="e2")
                            o2 = eo.tile([P, 512], dtf, tag="o2")
                            nc.scalar.dma_start(e2[0:64, :], qn[0:64, :])
                            nc.scalar.dma_start(e2[64:128, :], qn[0:64, :])
                            nc.sync.dma_start(o2[0:64, :], qn[64:128, :])
                            nc.sync.dma_start(o2[64:128, :], qn[64:128, :])
                            t1 = rt.tile([P, 512], dtf, tag="t1")
                            t2 = rt.tile([P, 512], dtf, tag="t2")
                            nc.vector.tensor_mul(t1[:], e2[:], c2_sb[:, ts(tq, 512)])
                            nc.vector.tensor_mul(t2[:], o2[:], s2_sb[:, ts(tq, 512)])
                            dest = QT[:, m, ts(tq, 512)] if m < HG else KT[:, ts(tq, 512)]
                            nc.vector.tensor_add(dest, t1[:], t2[:])
                        else:
                            vt = vst.tile([P, 512], dtf, tag="vt")
                            with tc.high_priority(offset=200):
                                nc.scalar.copy(vt[:], pss[m][:])
                            for i in range(4):
                                tp = vps.tile([P, P], dtf, tag="vtp")
                                nc.tensor.transpose(tp[:], vt[:, ts(i, P)], id_sb[:])
                                nc.scalar.copy(V[:, tq * 4 + i, :], tp[:])

            # ---------------- phase 2: attention + c_proj ------------------
            with (
                tc.tile_pool(name="stp", bufs=2, space="PSUM") as stp,
                tc.tile_pool(name="otp", bufs=2, space="PSUM") as otp,
                tc.tile_pool(name="rsp", bufs=2, space="PSUM") as rsp,
                tc.tile_pool(name="cpp", bufs=2, space="PSUM") as cpp,
                tc.tile_pool(name="ptp", bufs=6) as ptp,
                tc.tile_pool(name="smp", bufs=3) as smp,
                tc.tile_pool(name="ytp", bufs=8) as ytp,
                tc.tile_pool(name="rcpp", bufs=3) as rcpp,
                tc.tile_pool(name="obp", bufs=3) as obp,
            ):
                for qb in range(TQ):
                    yts = []
                    for h in range(HG):
                        nj = 4 * (qb + 1)
                        ot = otp.tile([P, 512], dtf, tag="ot")
                        rs = rsp.tile([P, 512], dtf, tag="rs")
                        for j in range(nj):
                            d = j - qb * 4
                            lo = d * P if d >= 0 else 0
                            st = stp.tile([P, 512], dtf, tag="st")
                            nc.tensor.matmul(
                                st[:, lo:512],
                                KT[:, ts(j, P)],
                                QT[:, h, qb * 512 + lo:(qb + 1) * 512],
                                start=True, stop=True,
                            )
                            pt = ptp.tile([P, 512], dtr, tag="pt")
                            if d >= 0:
                                # mask the triangular block in place, then one exp
                                nc.vector.tensor_add(
                                    st[:, lo:lo + P], st[:, lo:lo + P], tm_sb[:]
                                )
                                nc.scalar.activation(
                                    pt[:, lo:512], st[:, lo:512], FT.Exp, scale=SCALE
                                )
                            else:
                                nc.scalar.activation(pt[:], st[:], FT.Exp, scale=SCALE)
                            nc.tensor.matmul(
                                ot[:, lo:512], V[:, j, :], pt[:, lo:512],
                                start=(j == 0), stop=(j == nj - 1),
                            )
                            nc.tensor.matmul(
                                rs[:, lo:512], on_sb[:], pt[:, lo:512],
                                start=(j == 0), stop=(j == nj - 1),
                            )
                        rcp = rcpp.tile([P, 512], dtf, tag="rcp")
                        nc.vector.reciprocal_approx_fast(rcp[:], rs[:])
                        yt = ytp.tile([P, 512], dtr, tag="yt")
                        nc.vector.tensor_mul(yt[:], ot[:], rcp[:])
                        yts.append(yt)
                    for otile in range(16):
                        cp = cpp.tile([P, 512], dtf, tag="cp")
                        for h in range(HG):
                            nc.tensor.matmul(
                                cp[:], cw_sb[:, h, ts(otile, P)], yts[h][:],
                                start=(h == 0), stop=(h == HG - 1),
                            )
                        ob = obp.tile([P, 512], dtf, tag="ob")
                        nc.any.tensor_copy(ob[:], cp[:])
                        nc.sync.dma_start(outT[ts(otile, P), ts(qb, 512)], ob[:])

    nc.compile()
    return nc


def _get_nc():
    if "nc" not in _CACHE:
        _CACHE["nc"] = _build_bass()
    return _CACHE["nc"]


def _prep_core_inputs(x, q_w, kv_w, c_w):
    """Build the 8 per-core input dicts (core = 2*g + b)."""
    perm = np.concatenate([np.arange(0, P, 2), np.arange(1, P, 2)])  # even|odd

    # RoPE tables, stacked for the [64 even | 64 odd] block layout.
    pairs = np.arange(DK // 2, dtype=np.float64)
    freqs = 1.0 / ROPE_THETA ** (2.0 * pairs / DK)
    pos = np.arange(T, dtype=np.float64)
    ang = pos[None, :] * freqs[:, None]            # [64, T]
    cos = np.cos(ang)
    sin = np.sin(ang)
    c2 = np.concatenate([cos, sin], axis=0).astype(np.float32)      # [128, T]
    s2 = np.concatenate([-sin, cos], axis=0).astype(np.float32)     # [128, T]

    cc = np.arange(P)[:, None]
    qq = np.arange(P)[None, :]
    trimask = np.where(cc <= qq, 0.0, -1e30).astype(np.float32)
    ident = np.eye(P, dtype=np.float32)
    onesw = np.ones((P, P), dtype=np.float32)

    def chunk3(a2d):
        # [Dfull, F] -> [128, Dfull//128, F]
        d_full, f = a2d.shape
        return np.ascontiguousarray(
            a2d.reshape(d_full // P, P, f).transpose(1, 0, 2)
        )

    in_maps = []
    for core in range(8):
        g, b = core // 2, core % 2
        q_w_g = q_w[g * HG * DK:(g + 1) * HG * DK, :]          # [512, D]
        q_w_gp = q_w_g.reshape(HG, DK, D_MODEL)[:, perm, :].reshape(HG * DK, D_MODEL)
        k_w_g = kv_w[g * DK:(g + 1) * DK, :][perm, :]          # [128, D]
        v_w_g = kv_w[N_KV_HEADS * DK + g * DK: N_KV_HEADS * DK + (g + 1) * DK, :]
        c_w_g = c_w[:, g * HG * DK:(g + 1) * HG * DK]          # [D, 512]

        in_maps.append({
            "x3": chunk3(np.ascontiguousarray(x[b].T)),        # [128,16,T]
            "qw3": chunk3(np.ascontiguousarray(q_w_gp.T)),     # [128,16,512]
            "kw3": chunk3(np.ascontiguousarray(k_w_g.T)),      # [128,16,128]
            "vw3": chunk3(np.ascontiguousarray(v_w_g.T)),      # [128,16,128]
            "cw3": chunk3(np.ascontiguousarray(c_w_g.T)),      # [128,4,2048]
            "c2": c2, "s2": s2, "trimask": trimask,
            "ident": ident, "onesw": onesw,
        })
    return in_maps


def kernel(x, q_w, kv_w, c_w, _trace=False, _trace_cores=None):
    from concourse.bass_utils import run_bass_kernel_spmd

    nc = _get_nc()
    in_maps = _prep_core_inputs(
        np.asarray(x, dtype=np.float32), np.asarray(q_w, dtype=np.float32),
        np.asarray(kv_w, dtype=np.float32), np.asarray(c_w, dtype=np.float32),
    )
    res = run_bass_kernel_spmd(
        nc, in_maps, core_ids=list(range(8)),
        trace=_trace, trace_cores=_trace_cores,
    )
    _CACHE["last_results"] = res

    out = np.zeros((B, T, D_MODEL), dtype=np.float32)
    for core in range(8):
        g, b = core // 2, core % 2
        out[b] += res.results[core]["outT"].T
    return out
